# revision 19
# baseline (speedup 1.0000x reference)
"""TRN2 Bass kernel for nn_AutoRegressive (LSTM warmup + autoregressive decode).

Contract: kernel(**inputs) takes the FULL unsharded inputs
  inputs [2048, 48, 64], W [64, 4096], U [1024, 4096], b [4096],
  Wd [1024, 64], bd [64]
and returns the FULL output [2048, 64, 64] (float32), computed on 8
NeuronCores, data-parallel over the batch (256 rows per core).

Implementation notes:
- Transposed layout end-to-end: state hT [1024, 256] (units on partitions,
  batch on the free axis), so every matmul uses the weights in their natural
  layout as the stationary operand (out = lhsT.T @ rhs) and no on-chip
  transposes are needed anywhere.
- fp32r matmuls (1 cycle/row at N>=256, vs 4 for fp32). fp32r is fp32 with
  the low 12 mantissa bits zeroed; the multiply is exact given rounded
  inputs, accumulation is fp32 in PSUM. Inputs are pre-rounded on the host
  (RNE), on-chip producers (DVE/ACT) write fp32r directly.
- The bias b rides as row 64 of W_aug against a constant ones-row of the
  x operand; bd is applied by the DVE evacuation of the prediction.
- h is double-buffered across steps (z_t must read h_{t-1} while h_t is
  being written).
- Per step and unit, gate groups run in order i, f, g, o with PSUM banks
  [i,f | g] + [o], so the cell update (sigmoid/tanh on ACT, c/h on DVE)
  overlaps the later gate matmuls and the per-unit serial tail is short.
"""

import numpy as np

import concourse.mybir as mybir
import concourse.tile as tile
from concourse.bacc import Bacc
from concourse.bass_utils import run_bass_kernel_spmd

F32 = mybir.dt.float32
F32R = mybir.dt.float32r
BF16 = mybir.dt.bfloat16

B, T_IN, FEAT, UNITS, OUT_STEPS = 2048, 48, 64, 1024, 64
N_CORES = 8
BC = B // N_CORES  # 256
KT = UNITS // 128  # 8
GATE_N = 4 * UNITS  # 4096

SIG = mybir.ActivationFunctionType.Sigmoid
TANH = mybir.ActivationFunctionType.Tanh
IDENT = mybir.ActivationFunctionType.Identity


def to_f32r(a: np.ndarray) -> np.ndarray:
    """Round fp32 to fp32r (11 explicit mantissa bits, RNE). Bit-matches HW."""
    u = np.ascontiguousarray(a, dtype=np.float32).view(np.uint32)
    r = (u + np.uint32(0x7FF) + ((u >> np.uint32(12)) & np.uint32(1))) & np.uint32(
        0xFFFFF000
    )
    return r.view(np.float32)


def build_lstm(n_warm: int = T_IN, n_dec: int = OUT_STEPS, repeats: int = 1):
    """n_dec = number of outputs (first after warmup + n_dec-1 decode cells).

    repeats>1 re-runs the whole computation (including state zeroing)
    back-to-back inside one NEFF — used only for steady-state timing.
    """
    nc = Bacc("TRN2", target_bir_lowering=False)
    xt_d = nc.dram_tensor("xt", [n_warm, 65, BC], BF16, kind="ExternalInput")
    U_d = nc.dram_tensor("U", [128, KT, 4 * KT, 128], BF16, kind="ExternalInput")
    W_d = nc.dram_tensor("W", [65, GATE_N], BF16, kind="ExternalInput")
    Uf_d = nc.dram_tensor("Uf", [128, KT, 4 * KT, 128], BF16, kind="ExternalInput")
    Wd_d = nc.dram_tensor("Wd", [128, KT, FEAT], BF16, kind="ExternalInput")
    bd_d = nc.dram_tensor("bd", [FEAT, 1], F32, kind="ExternalInput")
    bf_d = nc.dram_tensor("bf", [128, 4 * KT], F32, kind="ExternalInput")
    out_d = nc.dram_tensor("outT", [n_dec, FEAT, BC], BF16, kind="ExternalOutput")

    n_steps = n_warm + (n_dec - 1)

    with tile.TileContext(nc) as tc:
        with (
            tc.tile_pool(name="weights", bufs=1) as wp,
            tc.tile_pool(name="state", bufs=1) as sp,
            tc.tile_pool(name="xs", bufs=4) as xp,
            tc.tile_pool(name="acts", bufs=3) as ap,
            tc.tile_pool(name="scratch", bufs=2) as scp,
            tc.tile_pool(name="psA", bufs=4, space="PSUM") as psa_p,
            tc.tile_pool(name="psO", bufs=3, space="PSUM") as pso_p,
            tc.tile_pool(name="psP", bufs=1, space="PSUM") as psp_p,
            tc.tile_pool(name="pbuf", bufs=2) as pb,
        ):
            U_sb = wp.tile([128, KT, 4 * KT, 128], BF16)
            Uf_sb = wp.tile([128, KT, 4 * KT, 128], BF16)
            W_sb = wp.tile([65, GATE_N], BF16)
            Wd_sb = wp.tile([128, KT, FEAT], BF16)
            bd_sb = wp.tile([FEAT, 1], F32)
            bf_sb = wp.tile([128, 4 * KT], F32)
            nc.sync.dma_start(out=W_sb[:, :], in_=W_d[:, :])
            # prefetch the first warm steps' x ahead of the 8MB U load so
            # step 0 is gated only on W (+x0), not the whole weight set
            n_pre = min(3, n_warm)
            x_pre = []
            for tp in range(n_pre):
                xtile = xp.tile([65, BC], BF16, tag="xstage")
                nc.sync.dma_start(out=xtile[:, :], in_=xt_d[tp, :, :])
                x_pre.append(xtile)
            # two DMAs per unit chunk -> unit 0's weights land in ~half the
            # single-queue time, so step-0 matmuls start earlier
            for uu in range(KT):
                nc.sync.dma_start(out=U_sb[:, uu, 0 : 2 * KT, :], in_=U_d[:, uu, 0 : 2 * KT, :])
                nc.sync.dma_start(out=U_sb[:, uu, 2 * KT :, :], in_=U_d[:, uu, 2 * KT :, :])
            nc.sync.dma_start(out=Wd_sb[:, :, :], in_=Wd_d[:, :, :])
            nc.sync.dma_start(out=bd_sb[:, :], in_=bd_d[:, :])

            # h double-buffered across steps: matmuls read bank t%2, the
            # h-update writes bank (t+1)%2 (z must use h from the previous step)
            h_k = [
                [
                    sp.tile([128, BC], BF16, name=f"h{bk}_{k}", tag=f"h{bk}_{k}")
                    for k in range(KT)
                ]
                for bk in range(2)
            ]
            c_k = [sp.tile([128, BC], F32, name=f"c{k}", tag=f"c{k}") for k in range(KT)]
            for rep in range(repeats):
              # no state zeroing: step 0 skips the U matmuls (h*U == 0) and
              # the f gate (c == 0), and writes c/h fresh, so h0/c0 are
              # never read
              for t in range(n_steps):
                h_rd = h_k[t % 2]
                h_wr = h_k[(t + 1) % 2]
                warm = t < n_warm
                if warm and t < n_pre and rep == 0:
                    x_rhs = x_pre[t]
                elif warm:
                    x_rhs = xp.tile([65, BC], BF16, tag="xstage")
                    nc.sync.dma_start(out=x_rhs[:, :], in_=xt_d[t, :, :])
                else:
                    x_rhs = None
                if t == 1 and rep == 0:
                    # folded decode weights: issued on the ACT hwdge queue
                    # after startup so they share DMA bandwidth only with
                    # the (tiny) per-step x loads; needed ~1.4ms from now
                    nc.scalar.dma_start(out=bf_sb[:, :], in_=bf_d[:, :])
                    for uu in range(KT):
                        nc.scalar.dma_start(
                            out=Uf_sb[:, uu, :, :], in_=Uf_d[:, uu, :, :]
                        )

                first = t == 0
                for u in range(KT):
                    # psA [128, 768]: cols [0:512] = bank A (i, f),
                    # cols [512:768] = bank B (g). o goes to its own psO bank.
                    # Group order i, f, g, o: the c-update chain starts right
                    # after f (overlapping g/o matmuls), so the post-matmul
                    # tail per unit is just sigmoid(o) + h-mul (~1us).
                    psA = psa_p.tile([128, 2 * BC], F32, tag="psA")
                    psGO = pso_p.tile([128, 2 * BC], F32, tag="psGO")

                    def group(out_ap, gi, w_first=False):
                        # at t=0 h is all-zero: the W matmul alone is z.
                        # decode steps use the folded recurrence U + Wd@W
                        # (x@W is absorbed), so they are pure U-chains.
                        zoff = gi * UNITS + u * 128
                        if warm and (w_first or first):
                            nc.tensor.matmul(
                                out_ap,
                                lhsT=W_sb[:, zoff : zoff + 128],
                                rhs=x_rhs[:, :],
                                start=True,
                                stop=first,
                            )
                        if first:
                            return
                        Um = U_sb if warm else Uf_sb
                        last_u = KT - 1
                        for kt in range(KT):
                            nc.tensor.matmul(
                                out_ap,
                                lhsT=Um[:, u, gi * KT + kt, :],
                                rhs=h_rd[kt][:, :],
                                start=(kt == 0 and not (warm and w_first)),
                                stop=(kt == last_u and (w_first or not warm)),
                            )
                        if warm and not w_first:
                            nc.tensor.matmul(
                                out_ap,
                                lhsT=W_sb[:, zoff : zoff + 128],
                                rhs=x_rhs[:, :],
                                start=False,
                                stop=True,
                            )

                    # in warm steps the x operand is ready early (DMA), so
                    # leading the first group with its W matmul gives the PE
                    # work while the previous step's h[7] chain completes
                    def bfv(gi):
                        # folded bias b + bd@W for decode-step gate gi
                        return bf_sb[:, gi * KT + u : gi * KT + u + 1]

                    group(psA[:, 0:BC], 0, w_first=(warm and u == 0))  # i
                    actA = ap.tile([128, 3 * BC], F32, tag="actA")
                    if not first and warm:
                        group(psA[:, BC : 2 * BC], 1)  # f
                        # sigmoid(i,f) fires once bank A is complete, while
                        # PE streams the g/o matmuls
                        nc.scalar.activation(
                            actA[:, 0 : 2 * BC], psA[:, 0 : 2 * BC], SIG
                        )
                        # c = sig(f)*c  (overlaps g matmuls)
                        nc.vector.tensor_mul(
                            c_k[u][:, :], actA[:, BC : 2 * BC], c_k[u][:, :]
                        )
                    elif not first:
                        # decode: per-gate activations carry the folded bias
                        nc.scalar.activation(
                            actA[:, 0:BC], psA[:, 0:BC], SIG, bias=bfv(0)
                        )
                        group(psA[:, BC : 2 * BC], 1)  # f
                        nc.scalar.activation(
                            actA[:, BC : 2 * BC], psA[:, BC : 2 * BC], SIG,
                            bias=bfv(1),
                        )
                        nc.vector.tensor_mul(
                            c_k[u][:, :], actA[:, BC : 2 * BC], c_k[u][:, :]
                        )
                    else:
                        nc.scalar.activation(actA[:, 0:BC], psA[:, 0:BC], SIG)

                    group(psGO[:, 0:BC], 2)  # g
                    if warm:
                        nc.scalar.activation(
                            actA[:, 2 * BC :], psGO[:, 0:BC], TANH
                        )
                    else:
                        nc.scalar.activation(
                            actA[:, 2 * BC :], psGO[:, 0:BC], TANH, bias=bfv(2)
                        )

                    group(psGO[:, BC : 2 * BC], 3)  # o
                    # evacuate sigmoid(o) immediately: releases the psGO
                    # bank as early as possible (later units' g-groups
                    # reuse it) and runs while DVE does the c-chain
                    actO = ap.tile([128, BC], F32, tag="actO")
                    if warm:
                        nc.scalar.activation(actO[:, :], psGO[:, BC : 2 * BC], SIG)
                    else:
                        nc.scalar.activation(
                            actO[:, :], psGO[:, BC : 2 * BC], SIG, bias=bfv(3)
                        )

                    tanc = scp.tile([128, BC], F32, tag="tanc")
                    if not first:
                        # c += sig(i)*tanh(g); tanh(c)
                        tmp = scp.tile([128, BC], F32, tag="tmp")
                        nc.vector.tensor_mul(
                            tmp[:, :], actA[:, 0:BC], actA[:, 2 * BC :]
                        )
                        nc.vector.tensor_add(c_k[u][:, :], c_k[u][:, :], tmp[:, :])
                    else:
                        # c0 == 0: c = sig(i)*tanh(g) is a fresh write
                        nc.vector.tensor_mul(
                            c_k[u][:, :], actA[:, 0:BC], actA[:, 2 * BC :]
                        )
                    nc.scalar.activation(tanc[:, :], c_k[u][:, :], TANH)
                    nc.vector.tensor_mul(h_wr[u][:, :], actO[:, :], tanc[:, :])

                # p-block: output prediction (also decode feedback)
                if t >= n_warm - 1:
                    psP = psp_p.tile([FEAT, BC], F32, tag="psP")
                    for kt in range(KT):
                        nc.tensor.matmul(
                            psP[:, :],
                            lhsT=Wd_sb[:, kt, :],
                            rhs=h_wr[kt][:, :],
                            start=(kt == 0),
                            stop=(kt == KT - 1),
                        )
                    # evacuate on ACT (identity with per-partition bias bd);
                    # off the critical path now that decode feedback is
                    # folded into Uf
                    pbuf = pb.tile([FEAT, BC], BF16, tag="pbuf")
                    nc.scalar.activation(
                        pbuf[:, :], psP[:, :], IDENT, bias=bd_sb[:, :]
                    )
                    nc.sync.dma_start(
                        out=out_d[t - (n_warm - 1), :, :],
                        in_=pbuf[:, :],
                    )
    nc.finalize()
    return nc


def to_bf16(a: np.ndarray):
    import ml_dtypes

    return np.ascontiguousarray(a).astype(ml_dtypes.bfloat16)


def _u_layout(U):
    # U[kt*128+p, gi*1024+u*128+c] -> [p, u, gi*KT+kt, c] (per-unit DMA chunks)
    return to_bf16(
        U.reshape(KT, 128, 4, KT, 128)
        .transpose(1, 3, 2, 0, 4)
        .reshape(128, KT, 4 * KT, 128)
    )


def prep_in_maps(inputs, W, U, b, Wd, bd, n_warm: int = T_IN):
    """Full unsharded reference inputs -> list of 8 per-core input maps."""
    W_aug = to_bf16(np.concatenate([W, b[None, :]], axis=0))  # [65, 4096]
    U_l = _u_layout(U)
    # decode recurrence fold: x_{t+1} = h_t@Wd + bd, so
    # z = x@W + h@U + b = h@(U + Wd@W) + (b + bd@W)
    Uf_l = _u_layout(
        (U + Wd.astype(np.float64) @ W.astype(np.float64)).astype(np.float32)
    )
    bfold = (b + bd.astype(np.float64) @ W.astype(np.float64)).astype(np.float32)
    # bf[gi*1024 + u*128 + p] -> [p, gi*KT+u]
    bf_l = np.ascontiguousarray(
        bfold.reshape(4, KT, 128).transpose(2, 0, 1).reshape(128, 4 * KT)
    )
    Wd_l = to_bf16(Wd.reshape(KT, 128, FEAT).transpose(1, 0, 2))  # [128, KT, 64]
    bd_l = np.ascontiguousarray(bd.astype(np.float32)[:, None])  # [64, 1]

    in_maps = []
    for c in range(N_CORES):
        xc = inputs[c * BC : (c + 1) * BC, :n_warm, :]  # [BC, T, F]
        xt = xc.transpose(1, 2, 0)  # [T, F, BC]
        xt_aug = np.concatenate(
            [xt, np.ones((n_warm, 1, BC), np.float32)], axis=1
        )  # [T, 65, BC]
        in_maps.append(
            {
                "xt": to_bf16(xt_aug),
                "U": U_l,
                "Uf": Uf_l,
                "W": W_aug,
                "Wd": Wd_l,
                "bd": bd_l,
                "bf": bf_l,
            }
        )
    return in_maps


def assemble_output(results, n_dec: int = OUT_STEPS):
    """Per-core outT [n_dec, 64, BC] -> full [B, n_dec, 64]."""
    outs = []
    for c in range(N_CORES):
        o = np.asarray(results[c]["outT"])  # [n_dec, FEAT, BC] bf16
        outs.append(o.transpose(2, 0, 1))  # [BC, n_dec, FEAT]
    return np.ascontiguousarray(np.concatenate(outs, axis=0).astype(np.float32))


_NC_CACHE = {}


def kernel(inputs, W, U, b, Wd, bd):
    inputs = np.asarray(inputs, dtype=np.float32)
    W = np.asarray(W, dtype=np.float32)
    U = np.asarray(U, dtype=np.float32)
    b = np.asarray(b, dtype=np.float32)
    Wd = np.asarray(Wd, dtype=np.float32)
    bd = np.asarray(bd, dtype=np.float32)
    assert inputs.shape == (B, T_IN, FEAT), inputs.shape

    if "nc" not in _NC_CACHE:
        _NC_CACHE["nc"] = build_lstm(T_IN, OUT_STEPS)
    nc = _NC_CACHE["nc"]

    in_maps = prep_in_maps(inputs, W, U, b, Wd, bd)
    res = run_bass_kernel_spmd(nc, in_maps, core_ids=list(range(N_CORES)))
    return assemble_output(res.results)



# revision 20
# speedup vs baseline: 1.0520x; 1.0520x over previous
"""TRN2 Bass kernel for nn_AutoRegressive (LSTM warmup + autoregressive decode).

Contract: kernel(**inputs) takes the FULL unsharded inputs
  inputs [2048, 48, 64], W [64, 4096], U [1024, 4096], b [4096],
  Wd [1024, 64], bd [64]
and returns the FULL output [2048, 64, 64] (float32), computed on 8
NeuronCores, data-parallel over the batch (256 rows per core).

Implementation notes:
- Transposed layout end-to-end: state hT [1024, 256] (units on partitions,
  batch on the free axis), so every matmul uses the weights in their natural
  layout as the stationary operand (out = lhsT.T @ rhs) and no on-chip
  transposes are needed anywhere.
- All matmul operands are bf16 (1 PE cycle/row, and unlike fp32/fp32r
  weights, bf16 enables the HW fast-weight-load path); accumulation is
  fp32 in PSUM, the cell state c stays fp32. Final rel err ~4e-3 vs the
  2e-2 gate.
- Decode feedback is folded into the recurrence: x_{t+1} = h_t@Wd + bd
  implies z = h@(U + Wd@W) + (b + bd@W), so decode steps run pure
  8-matmul U-chains per gate (no x@W matmuls, no feedback dependency);
  the folded bias enters via the ACT activations' per-partition bias
  operand. The prediction head h@Wd only feeds the output DMA.
- Warm-step bias b rides as row 64 of W_aug against the ones-row of the
  DMA-staged x; step 0 skips all U matmuls and the f gate (h0 = c0 = 0),
  which also removes any state zeroing.
- h is double-buffered across steps (z_t must read h_{t-1} while h_t is
  being written).
- Per step and unit, gate groups run i, f, g, o with PSUM pools
  psIF[i|f] (bufs=4) and psGO[g|o] (bufs=3) + psP (1) = 8 banks; the
  cell update (sigmoid/tanh on ACT, c/h on DVE) overlaps later gate
  matmuls, sigmoid(o) is evacuated right after the o-group so the bank
  recycles early, and the PE stays ~96% busy in steady state.
"""

import numpy as np

import concourse.mybir as mybir
import concourse.tile as tile
from concourse.bacc import Bacc
from concourse.bass_utils import run_bass_kernel_spmd

F32 = mybir.dt.float32
F32R = mybir.dt.float32r
BF16 = mybir.dt.bfloat16

B, T_IN, FEAT, UNITS, OUT_STEPS = 2048, 48, 64, 1024, 64
N_CORES = 8
BC = B // N_CORES  # 256
KT = UNITS // 128  # 8
GATE_N = 4 * UNITS  # 4096

SIG = mybir.ActivationFunctionType.Sigmoid
TANH = mybir.ActivationFunctionType.Tanh
IDENT = mybir.ActivationFunctionType.Identity


def to_f32r(a: np.ndarray) -> np.ndarray:
    """Round fp32 to fp32r (11 explicit mantissa bits, RNE). Bit-matches HW."""
    u = np.ascontiguousarray(a, dtype=np.float32).view(np.uint32)
    r = (u + np.uint32(0x7FF) + ((u >> np.uint32(12)) & np.uint32(1))) & np.uint32(
        0xFFFFF000
    )
    return r.view(np.float32)


def build_lstm(n_warm: int = T_IN, n_dec: int = OUT_STEPS, repeats: int = 1):
    """n_dec = number of outputs (first after warmup + n_dec-1 decode cells).

    repeats>1 re-runs the whole computation (including state zeroing)
    back-to-back inside one NEFF — used only for steady-state timing.
    """
    nc = Bacc("TRN2", target_bir_lowering=False)
    xt_d = nc.dram_tensor("xt", [n_warm, 65, BC], BF16, kind="ExternalInput")
    U_d = nc.dram_tensor("U", [128, KT, 4 * KT, 128], BF16, kind="ExternalInput")
    W_d = nc.dram_tensor("W", [65, GATE_N], BF16, kind="ExternalInput")
    Uf_d = nc.dram_tensor("Uf", [128, KT, 4 * KT, 128], BF16, kind="ExternalInput")
    Wd_d = nc.dram_tensor("Wd", [128, KT, FEAT], BF16, kind="ExternalInput")
    bd_d = nc.dram_tensor("bd", [FEAT, 1], F32, kind="ExternalInput")
    bf_d = nc.dram_tensor("bf", [128, 4 * KT], F32, kind="ExternalInput")
    out_d = nc.dram_tensor("outT", [n_dec, FEAT, BC], BF16, kind="ExternalOutput")

    n_steps = n_warm + (n_dec - 1)

    with tile.TileContext(nc) as tc:
        with (
            tc.tile_pool(name="weights", bufs=1) as wp,
            tc.tile_pool(name="state", bufs=1) as sp,
            tc.tile_pool(name="xs", bufs=4) as xp,
            tc.tile_pool(name="acts", bufs=3) as ap,
            tc.tile_pool(name="scratch", bufs=2) as scp,
            tc.tile_pool(name="psA", bufs=4, space="PSUM") as psa_p,
            tc.tile_pool(name="psO", bufs=3, space="PSUM") as pso_p,
            tc.tile_pool(name="psP", bufs=1, space="PSUM") as psp_p,
            tc.tile_pool(name="pbuf", bufs=2) as pb,
        ):
            U_sb = wp.tile([128, KT, 4 * KT, 128], BF16)
            Uf_sb = wp.tile([128, KT, 4 * KT, 128], BF16)
            W_sb = wp.tile([65, GATE_N], BF16)
            Wd_sb = wp.tile([128, KT, FEAT], BF16)
            bd_sb = wp.tile([FEAT, 1], F32)
            bf_sb = wp.tile([128, 4 * KT], F32)
            nc.sync.dma_start(out=W_sb[:, :], in_=W_d[:, :])
            # prefetch the first warm steps' x ahead of the 8MB U load so
            # step 0 is gated only on W (+x0), not the whole weight set
            n_pre = min(3, n_warm)
            x_pre = []
            for tp in range(n_pre):
                xtile = xp.tile([65, BC], BF16, tag="xstage")
                nc.sync.dma_start(out=xtile[:, :], in_=xt_d[tp, :, :])
                x_pre.append(xtile)
            # two DMAs per unit chunk -> unit 0's weights land in ~half the
            # single-queue time, so step-0 matmuls start earlier
            for uu in range(KT):
                nc.sync.dma_start(out=U_sb[:, uu, 0 : 2 * KT, :], in_=U_d[:, uu, 0 : 2 * KT, :])
                nc.sync.dma_start(out=U_sb[:, uu, 2 * KT :, :], in_=U_d[:, uu, 2 * KT :, :])
            nc.sync.dma_start(out=Wd_sb[:, :, :], in_=Wd_d[:, :, :])
            nc.sync.dma_start(out=bd_sb[:, :], in_=bd_d[:, :])

            # h double-buffered across steps: matmuls read bank t%2, the
            # h-update writes bank (t+1)%2 (z must use h from the previous step)
            h_k = [
                [
                    sp.tile([128, BC], BF16, name=f"h{bk}_{k}", tag=f"h{bk}_{k}")
                    for k in range(KT)
                ]
                for bk in range(2)
            ]
            c_k = [sp.tile([128, BC], F32, name=f"c{k}", tag=f"c{k}") for k in range(KT)]
            for rep in range(repeats):
              # no state zeroing: step 0 skips the U matmuls (h*U == 0) and
              # the f gate (c == 0), and writes c/h fresh, so h0/c0 are
              # never read
              for t in range(n_steps):
                h_rd = h_k[t % 2]
                h_wr = h_k[(t + 1) % 2]
                warm = t < n_warm
                if warm and t < n_pre and rep == 0:
                    x_rhs = x_pre[t]
                elif warm:
                    x_rhs = xp.tile([65, BC], BF16, tag="xstage")
                    nc.sync.dma_start(out=x_rhs[:, :], in_=xt_d[t, :, :])
                else:
                    x_rhs = None
                if t == 1 and rep == 0:
                    # folded decode weights: issued on the ACT hwdge queue
                    # after startup so they share DMA bandwidth only with
                    # the (tiny) per-step x loads; needed ~1.4ms from now
                    nc.scalar.dma_start(out=bf_sb[:, :], in_=bf_d[:, :])
                    for uu in range(KT):
                        nc.scalar.dma_start(
                            out=Uf_sb[:, uu, :, :], in_=Uf_d[:, uu, :, :]
                        )

                first = t == 0
                for u in range(KT):
                    # psA [128, 768]: cols [0:512] = bank A (i, f),
                    # cols [512:768] = bank B (g). o goes to its own psO bank.
                    # Group order i, f, g, o: the c-update chain starts right
                    # after f (overlapping g/o matmuls), so the post-matmul
                    # tail per unit is just sigmoid(o) + h-mul (~1us).
                    psA = psa_p.tile([128, 2 * BC], F32, tag="psA")
                    psGO = pso_p.tile([128, 2 * BC], F32, tag="psGO")

                    def group(out_ap, gi, w_first=False):
                        # at t=0 h is all-zero: the W matmul alone is z.
                        # decode steps use the folded recurrence U + Wd@W
                        # (x@W is absorbed), so they are pure U-chains.
                        zoff = gi * UNITS + u * 128
                        if warm and (w_first or first):
                            nc.tensor.matmul(
                                out_ap,
                                lhsT=W_sb[:, zoff : zoff + 128],
                                rhs=x_rhs[:, :],
                                start=True,
                                stop=first,
                            )
                        if first:
                            return
                        Um = U_sb if warm else Uf_sb
                        last_u = KT - 1
                        for kt in range(KT):
                            nc.tensor.matmul(
                                out_ap,
                                lhsT=Um[:, u, gi * KT + kt, :],
                                rhs=h_rd[kt][:, :],
                                start=(kt == 0 and not (warm and w_first)),
                                stop=(kt == last_u and (w_first or not warm)),
                            )
                        if warm and not w_first:
                            nc.tensor.matmul(
                                out_ap,
                                lhsT=W_sb[:, zoff : zoff + 128],
                                rhs=x_rhs[:, :],
                                start=False,
                                stop=True,
                            )

                    # in warm steps the x operand is ready early (DMA), so
                    # leading the first group with its W matmul gives the PE
                    # work while the previous step's h[7] chain completes
                    def bfv(gi):
                        # folded bias b + bd@W for decode-step gate gi
                        return bf_sb[:, gi * KT + u : gi * KT + u + 1]

                    group(psA[:, 0:BC], 0, w_first=(warm and u == 0))  # i
                    actA = ap.tile([128, 3 * BC], F32, tag="actA")
                    if not first and warm:
                        group(psA[:, BC : 2 * BC], 1)  # f
                        # sigmoid(i,f) fires once bank A is complete, while
                        # PE streams the g/o matmuls
                        nc.scalar.activation(
                            actA[:, 0 : 2 * BC], psA[:, 0 : 2 * BC], SIG
                        )
                        # c = sig(f)*c  (overlaps g matmuls)
                        nc.vector.tensor_mul(
                            c_k[u][:, :], actA[:, BC : 2 * BC], c_k[u][:, :]
                        )
                    elif not first:
                        # decode: per-gate activations carry the folded bias
                        nc.scalar.activation(
                            actA[:, 0:BC], psA[:, 0:BC], SIG, bias=bfv(0)
                        )
                        group(psA[:, BC : 2 * BC], 1)  # f
                        nc.scalar.activation(
                            actA[:, BC : 2 * BC], psA[:, BC : 2 * BC], SIG,
                            bias=bfv(1),
                        )
                        nc.vector.tensor_mul(
                            c_k[u][:, :], actA[:, BC : 2 * BC], c_k[u][:, :]
                        )
                    else:
                        nc.scalar.activation(actA[:, 0:BC], psA[:, 0:BC], SIG)

                    group(psGO[:, 0:BC], 2)  # g
                    if warm:
                        nc.scalar.activation(
                            actA[:, 2 * BC :], psGO[:, 0:BC], TANH
                        )
                    else:
                        nc.scalar.activation(
                            actA[:, 2 * BC :], psGO[:, 0:BC], TANH, bias=bfv(2)
                        )

                    group(psGO[:, BC : 2 * BC], 3)  # o
                    # evacuate sigmoid(o) immediately: releases the psGO
                    # bank as early as possible (later units' g-groups
                    # reuse it) and runs while DVE does the c-chain
                    actO = ap.tile([128, BC], F32, tag="actO")
                    if warm:
                        nc.scalar.activation(actO[:, :], psGO[:, BC : 2 * BC], SIG)
                    else:
                        nc.scalar.activation(
                            actO[:, :], psGO[:, BC : 2 * BC], SIG, bias=bfv(3)
                        )

                    tanc = scp.tile([128, BC], F32, tag="tanc")
                    if not first:
                        # c += sig(i)*tanh(g); tanh(c)
                        tmp = scp.tile([128, BC], F32, tag="tmp")
                        nc.vector.tensor_mul(
                            tmp[:, :], actA[:, 0:BC], actA[:, 2 * BC :]
                        )
                        nc.vector.tensor_add(c_k[u][:, :], c_k[u][:, :], tmp[:, :])
                    else:
                        # c0 == 0: c = sig(i)*tanh(g) is a fresh write
                        nc.vector.tensor_mul(
                            c_k[u][:, :], actA[:, 0:BC], actA[:, 2 * BC :]
                        )
                    nc.scalar.activation(tanc[:, :], c_k[u][:, :], TANH)
                    nc.vector.tensor_mul(h_wr[u][:, :], actO[:, :], tanc[:, :])

                # p-block: output prediction (also decode feedback)
                if t >= n_warm - 1:
                    psP = psp_p.tile([FEAT, BC], F32, tag="psP")
                    for kt in range(KT):
                        nc.tensor.matmul(
                            psP[:, :],
                            lhsT=Wd_sb[:, kt, :],
                            rhs=h_wr[kt][:, :],
                            start=(kt == 0),
                            stop=(kt == KT - 1),
                        )
                    # evacuate on ACT (identity with per-partition bias bd);
                    # off the critical path now that decode feedback is
                    # folded into Uf
                    pbuf = pb.tile([FEAT, BC], BF16, tag="pbuf")
                    nc.scalar.activation(
                        pbuf[:, :], psP[:, :], IDENT, bias=bd_sb[:, :]
                    )
                    nc.sync.dma_start(
                        out=out_d[t - (n_warm - 1), :, :],
                        in_=pbuf[:, :],
                    )
    nc.finalize()
    return nc


def to_bf16(a: np.ndarray):
    import ml_dtypes

    return np.ascontiguousarray(a).astype(ml_dtypes.bfloat16)


def _u_layout(U):
    # U[kt*128+p, gi*1024+u*128+c] -> [p, u, gi*KT+kt, c] (per-unit DMA chunks)
    return to_bf16(
        U.reshape(KT, 128, 4, KT, 128)
        .transpose(1, 3, 2, 0, 4)
        .reshape(128, KT, 4 * KT, 128)
    )


def prep_in_maps(inputs, W, U, b, Wd, bd, n_warm: int = T_IN):
    """Full unsharded reference inputs -> list of 8 per-core input maps."""
    W_aug = to_bf16(np.concatenate([W, b[None, :]], axis=0))  # [65, 4096]
    U_l = _u_layout(U)
    # decode recurrence fold: x_{t+1} = h_t@Wd + bd, so
    # z = x@W + h@U + b = h@(U + Wd@W) + (b + bd@W)
    Uf_l = _u_layout(
        (U + Wd.astype(np.float64) @ W.astype(np.float64)).astype(np.float32)
    )
    bfold = (b + bd.astype(np.float64) @ W.astype(np.float64)).astype(np.float32)
    # bf[gi*1024 + u*128 + p] -> [p, gi*KT+u]
    bf_l = np.ascontiguousarray(
        bfold.reshape(4, KT, 128).transpose(2, 0, 1).reshape(128, 4 * KT)
    )
    Wd_l = to_bf16(Wd.reshape(KT, 128, FEAT).transpose(1, 0, 2))  # [128, KT, 64]
    bd_l = np.ascontiguousarray(bd.astype(np.float32)[:, None])  # [64, 1]

    in_maps = []
    for c in range(N_CORES):
        xc = inputs[c * BC : (c + 1) * BC, :n_warm, :]  # [BC, T, F]
        xt = xc.transpose(1, 2, 0)  # [T, F, BC]
        xt_aug = np.concatenate(
            [xt, np.ones((n_warm, 1, BC), np.float32)], axis=1
        )  # [T, 65, BC]
        in_maps.append(
            {
                "xt": to_bf16(xt_aug),
                "U": U_l,
                "Uf": Uf_l,
                "W": W_aug,
                "Wd": Wd_l,
                "bd": bd_l,
                "bf": bf_l,
            }
        )
    return in_maps


def assemble_output(results, n_dec: int = OUT_STEPS):
    """Per-core outT [n_dec, 64, BC] -> full [B, n_dec, 64]."""
    outs = []
    for c in range(N_CORES):
        o = np.asarray(results[c]["outT"])  # [n_dec, FEAT, BC] bf16
        outs.append(o.transpose(2, 0, 1))  # [BC, n_dec, FEAT]
    return np.ascontiguousarray(np.concatenate(outs, axis=0).astype(np.float32))


_NC_CACHE = {}


def kernel(inputs, W, U, b, Wd, bd):
    inputs = np.asarray(inputs, dtype=np.float32)
    W = np.asarray(W, dtype=np.float32)
    U = np.asarray(U, dtype=np.float32)
    b = np.asarray(b, dtype=np.float32)
    Wd = np.asarray(Wd, dtype=np.float32)
    bd = np.asarray(bd, dtype=np.float32)
    assert inputs.shape == (B, T_IN, FEAT), inputs.shape

    if "nc" not in _NC_CACHE:
        _NC_CACHE["nc"] = build_lstm(T_IN, OUT_STEPS)
    nc = _NC_CACHE["nc"]

    in_maps = prep_in_maps(inputs, W, U, b, Wd, bd)
    res = run_bass_kernel_spmd(nc, in_maps, core_ids=list(range(N_CORES)))
    return assemble_output(res.results)



# revision 22
# speedup vs baseline: 1.0580x; 1.0057x over previous
"""TRN2 Bass kernel for nn_AutoRegressive (LSTM warmup + autoregressive decode).

Contract: kernel(**inputs) takes the FULL unsharded inputs
  inputs [2048, 48, 64], W [64, 4096], U [1024, 4096], b [4096],
  Wd [1024, 64], bd [64]
and returns the FULL output [2048, 64, 64] (float32), computed on 8
NeuronCores, data-parallel over the batch (256 rows per core).

Implementation notes:
- Transposed layout end-to-end: state hT [1024, 256] (units on partitions,
  batch on the free axis), so every matmul uses the weights in their natural
  layout as the stationary operand (out = lhsT.T @ rhs) and no on-chip
  transposes are needed anywhere.
- All matmul operands are bf16 (1 PE cycle/row, and unlike fp32/fp32r
  weights, bf16 enables the HW fast-weight-load path); accumulation is
  fp32 in PSUM, the cell state c stays fp32. Final rel err ~4e-3 vs the
  2e-2 gate.
- Decode feedback is folded into the recurrence: x_{t+1} = h_t@Wd + bd
  implies z = h@(U + Wd@W) + (b + bd@W), so decode steps run pure
  8-matmul U-chains per gate (no x@W matmuls, no feedback dependency);
  the folded bias enters via the ACT activations' per-partition bias
  operand. The prediction head h@Wd only feeds the output DMA.
- Warm-step bias b rides as row 64 of W_aug against the ones-row of the
  DMA-staged x; step 0 skips all U matmuls and the f gate (h0 = c0 = 0),
  which also removes any state zeroing.
- h is double-buffered across steps (z_t must read h_{t-1} while h_t is
  being written).
- Per step and unit, gate groups run i, f, g, o with PSUM pools
  psIF[i|f] (bufs=4) and psGO[g|o] (bufs=3) + psP (1) = 8 banks; the
  cell update (sigmoid/tanh on ACT, c/h on DVE) overlaps later gate
  matmuls, sigmoid(o) is evacuated right after the o-group so the bank
  recycles early, and the PE stays ~96% busy in steady state.
"""

import numpy as np

import concourse.mybir as mybir
import concourse.tile as tile
from concourse.bacc import Bacc
from concourse.bass_utils import run_bass_kernel_spmd

F32 = mybir.dt.float32
F32R = mybir.dt.float32r
BF16 = mybir.dt.bfloat16

B, T_IN, FEAT, UNITS, OUT_STEPS = 2048, 48, 64, 1024, 64
N_CORES = 8
BC = B // N_CORES  # 256
KT = UNITS // 128  # 8
GATE_N = 4 * UNITS  # 4096

SIG = mybir.ActivationFunctionType.Sigmoid
TANH = mybir.ActivationFunctionType.Tanh
IDENT = mybir.ActivationFunctionType.Identity


def to_f32r(a: np.ndarray) -> np.ndarray:
    """Round fp32 to fp32r (11 explicit mantissa bits, RNE). Bit-matches HW."""
    u = np.ascontiguousarray(a, dtype=np.float32).view(np.uint32)
    r = (u + np.uint32(0x7FF) + ((u >> np.uint32(12)) & np.uint32(1))) & np.uint32(
        0xFFFFF000
    )
    return r.view(np.float32)


def build_lstm(n_warm: int = T_IN, n_dec: int = OUT_STEPS, repeats: int = 1,
               use_bias: bool = True):
    """n_dec = number of outputs (first after warmup + n_dec-1 decode cells).

    repeats>1 re-runs the whole computation (including state zeroing)
    back-to-back inside one NEFF — used only for steady-state timing.
    """
    nc = Bacc("TRN2", target_bir_lowering=False)
    xt_d = nc.dram_tensor("xt", [n_warm, 65, BC], BF16, kind="ExternalInput")
    U_d = nc.dram_tensor("U", [128, KT, 4 * KT, 128], BF16, kind="ExternalInput")
    W_d = nc.dram_tensor("W", [65, GATE_N], BF16, kind="ExternalInput")
    Uf_d = nc.dram_tensor("Uf", [128, KT, 4 * KT, 128], BF16, kind="ExternalInput")
    Wd_d = nc.dram_tensor("Wd", [128, KT, FEAT], BF16, kind="ExternalInput")
    bd_d = nc.dram_tensor("bd", [FEAT, 1], F32, kind="ExternalInput")
    bf_d = nc.dram_tensor("bf", [128, 4 * KT], F32, kind="ExternalInput")
    out_d = nc.dram_tensor("outT", [n_dec, FEAT, BC], BF16, kind="ExternalOutput")

    n_steps = n_warm + (n_dec - 1)

    with tile.TileContext(nc) as tc:
        with (
            tc.tile_pool(name="weights", bufs=1) as wp,
            tc.tile_pool(name="state", bufs=1) as sp,
            tc.tile_pool(name="xs", bufs=4) as xp,
            tc.tile_pool(name="acts", bufs=3) as ap,
            tc.tile_pool(name="scratch", bufs=2) as scp,
            tc.tile_pool(name="psA", bufs=3, space="PSUM") as psa_p,
            tc.tile_pool(name="psG", bufs=2, space="PSUM") as psg_p,
            tc.tile_pool(name="psO", bufs=2, space="PSUM") as pso_p,
            tc.tile_pool(name="psP", bufs=1, space="PSUM") as psp_p,
            tc.tile_pool(name="pbuf", bufs=2) as pb,
        ):
            U_sb = wp.tile([128, KT, 4 * KT, 128], BF16)
            Uf_sb = wp.tile([128, KT, 4 * KT, 128], BF16)
            W_sb = wp.tile([65, GATE_N], BF16)
            Wd_sb = wp.tile([128, KT, FEAT], BF16)
            bd_sb = wp.tile([FEAT, 1], F32)
            bf_sb = wp.tile([128, 4 * KT], F32)
            nc.sync.dma_start(out=W_sb[:, :], in_=W_d[:, :])
            # prefetch the first warm steps' x ahead of the 8MB U load so
            # step 0 is gated only on W (+x0), not the whole weight set
            n_pre = min(3, n_warm)
            x_pre = []
            for tp in range(n_pre):
                xtile = xp.tile([65, BC], BF16, tag="xstage")
                nc.sync.dma_start(out=xtile[:, :], in_=xt_d[tp, :, :])
                x_pre.append(xtile)
            # two DMAs per unit chunk -> unit 0's weights land in ~half the
            # single-queue time, so step-0 matmuls start earlier
            for uu in range(KT):
                nc.sync.dma_start(out=U_sb[:, uu, 0 : 2 * KT, :], in_=U_d[:, uu, 0 : 2 * KT, :])
                nc.sync.dma_start(out=U_sb[:, uu, 2 * KT :, :], in_=U_d[:, uu, 2 * KT :, :])
            nc.sync.dma_start(out=Wd_sb[:, :, :], in_=Wd_d[:, :, :])
            nc.sync.dma_start(out=bd_sb[:, :], in_=bd_d[:, :])

            # h double-buffered across steps: matmuls read bank t%2, the
            # h-update writes bank (t+1)%2 (z must use h from the previous step)
            h_k = [
                [
                    sp.tile([128, BC], BF16, name=f"h{bk}_{k}", tag=f"h{bk}_{k}")
                    for k in range(KT)
                ]
                for bk in range(2)
            ]
            c_k = [sp.tile([128, BC], F32, name=f"c{k}", tag=f"c{k}") for k in range(KT)]
            for rep in range(repeats):
              # no state zeroing: step 0 skips the U matmuls (h*U == 0) and
              # the f gate (c == 0), and writes c/h fresh, so h0/c0 are
              # never read
              for t in range(n_steps):
                h_rd = h_k[t % 2]
                h_wr = h_k[(t + 1) % 2]
                warm = t < n_warm
                if warm and t < n_pre and rep == 0:
                    x_rhs = x_pre[t]
                elif warm:
                    x_rhs = xp.tile([65, BC], BF16, tag="xstage")
                    nc.sync.dma_start(out=x_rhs[:, :], in_=xt_d[t, :, :])
                else:
                    x_rhs = None
                if t == 1 and rep == 0:
                    # folded decode weights: issued on the ACT hwdge queue
                    # after startup so they share DMA bandwidth only with
                    # the (tiny) per-step x loads; needed ~1.4ms from now
                    nc.scalar.dma_start(out=bf_sb[:, :], in_=bf_d[:, :])
                    for uu in range(KT):
                        nc.scalar.dma_start(
                            out=Uf_sb[:, uu, :, :], in_=Uf_d[:, uu, :, :]
                        )

                first = t == 0
                for u in range(KT):
                    # psA [128, 768]: cols [0:512] = bank A (i, f),
                    # cols [512:768] = bank B (g). o goes to its own psO bank.
                    # Group order i, f, g, o: the c-update chain starts right
                    # after f (overlapping g/o matmuls), so the post-matmul
                    # tail per unit is just sigmoid(o) + h-mul (~1us).
                    psA = psa_p.tile([128, 2 * BC], F32, tag="psA")
                    psG = psg_p.tile([128, BC], F32, tag="psG")
                    psO = pso_p.tile([128, BC], F32, tag="psO")

                    def group(out_ap, gi, w_first=False):
                        # at t=0 h is all-zero: the W matmul alone is z.
                        # decode steps use the folded recurrence U + Wd@W
                        # (x@W is absorbed), so they are pure U-chains.
                        zoff = gi * UNITS + u * 128
                        if warm and (w_first or first):
                            nc.tensor.matmul(
                                out_ap,
                                lhsT=W_sb[:, zoff : zoff + 128],
                                rhs=x_rhs[:, :],
                                start=True,
                                stop=first,
                            )
                        if first:
                            return
                        Um = U_sb if warm else Uf_sb
                        last_u = KT - 1
                        for kt in range(KT):
                            nc.tensor.matmul(
                                out_ap,
                                lhsT=Um[:, u, gi * KT + kt, :],
                                rhs=h_rd[kt][:, :],
                                start=(kt == 0 and not (warm and w_first)),
                                stop=(kt == last_u and (w_first or not warm)),
                            )
                        if warm and not w_first:
                            nc.tensor.matmul(
                                out_ap,
                                lhsT=W_sb[:, zoff : zoff + 128],
                                rhs=x_rhs[:, :],
                                start=False,
                                stop=True,
                            )

                    # in warm steps the x operand is ready early (DMA), so
                    # leading the first group with its W matmul gives the PE
                    # work while the previous step's h[7] chain completes
                    def bfv(gi):
                        # folded bias b + bd@W for decode-step gate gi
                        return bf_sb[:, gi * KT + u : gi * KT + u + 1]

                    group(psA[:, 0:BC], 0, w_first=(warm and u == 0))  # i
                    actA = ap.tile([128, 3 * BC], F32, tag="actA")
                    if not first and (warm or not use_bias):
                        group(psA[:, BC : 2 * BC], 1)  # f
                        # sigmoid(i,f) fires once bank A is complete, while
                        # PE streams the g/o matmuls
                        nc.scalar.activation(
                            actA[:, 0 : 2 * BC], psA[:, 0 : 2 * BC], SIG
                        )
                        # c = sig(f)*c  (overlaps g matmuls)
                        nc.vector.tensor_mul(
                            c_k[u][:, :], actA[:, BC : 2 * BC], c_k[u][:, :]
                        )
                    elif not first:
                        # decode: per-gate activations carry the folded bias
                        nc.scalar.activation(
                            actA[:, 0:BC], psA[:, 0:BC], SIG, bias=bfv(0)
                        )
                        group(psA[:, BC : 2 * BC], 1)  # f
                        nc.scalar.activation(
                            actA[:, BC : 2 * BC], psA[:, BC : 2 * BC], SIG,
                            bias=bfv(1),
                        )
                        nc.vector.tensor_mul(
                            c_k[u][:, :], actA[:, BC : 2 * BC], c_k[u][:, :]
                        )
                    else:
                        nc.scalar.activation(actA[:, 0:BC], psA[:, 0:BC], SIG)

                    group(psG[:, :], 2)  # g
                    if warm or not use_bias:
                        nc.scalar.activation(
                            actA[:, 2 * BC :], psG[:, :], TANH
                        )
                    else:
                        nc.scalar.activation(
                            actA[:, 2 * BC :], psG[:, :], TANH, bias=bfv(2)
                        )

                    group(psO[:, :], 3)  # o
                    # evacuate sigmoid(o) immediately: releases the psGO
                    # bank as early as possible (later units' g-groups
                    # reuse it) and runs while DVE does the c-chain
                    actO = ap.tile([128, BC], F32, tag="actO")
                    if warm or not use_bias:
                        nc.scalar.activation(actO[:, :], psO[:, :], SIG)
                    else:
                        nc.scalar.activation(
                            actO[:, :], psO[:, :], SIG, bias=bfv(3)
                        )

                    tanc = scp.tile([128, BC], F32, tag="tanc")
                    if not first:
                        # c += sig(i)*tanh(g); tanh(c)
                        tmp = scp.tile([128, BC], F32, tag="tmp")
                        nc.vector.tensor_mul(
                            tmp[:, :], actA[:, 0:BC], actA[:, 2 * BC :]
                        )
                        nc.vector.tensor_add(c_k[u][:, :], c_k[u][:, :], tmp[:, :])
                    else:
                        # c0 == 0: c = sig(i)*tanh(g) is a fresh write
                        nc.vector.tensor_mul(
                            c_k[u][:, :], actA[:, 0:BC], actA[:, 2 * BC :]
                        )
                    nc.scalar.activation(tanc[:, :], c_k[u][:, :], TANH)
                    nc.vector.tensor_mul(h_wr[u][:, :], actO[:, :], tanc[:, :])

                # p-block: output prediction (also decode feedback)
                if t >= n_warm - 1:
                    psP = psp_p.tile([FEAT, BC], F32, tag="psP")
                    for kt in range(KT):
                        nc.tensor.matmul(
                            psP[:, :],
                            lhsT=Wd_sb[:, kt, :],
                            rhs=h_wr[kt][:, :],
                            start=(kt == 0),
                            stop=(kt == KT - 1),
                        )
                    # evacuate on ACT (identity with per-partition bias bd);
                    # off the critical path now that decode feedback is
                    # folded into Uf
                    pbuf = pb.tile([FEAT, BC], BF16, tag="pbuf")
                    nc.scalar.activation(
                        pbuf[:, :], psP[:, :], IDENT, bias=bd_sb[:, :]
                    )
                    nc.sync.dma_start(
                        out=out_d[t - (n_warm - 1), :, :],
                        in_=pbuf[:, :],
                    )
    nc.finalize()
    return nc


def to_bf16(a: np.ndarray):
    import ml_dtypes

    return np.ascontiguousarray(a).astype(ml_dtypes.bfloat16)


def _u_layout(U):
    # U[kt*128+p, gi*1024+u*128+c] -> [p, u, gi*KT+kt, c] (per-unit DMA chunks)
    return to_bf16(
        U.reshape(KT, 128, 4, KT, 128)
        .transpose(1, 3, 2, 0, 4)
        .reshape(128, KT, 4 * KT, 128)
    )


def prep_in_maps(inputs, W, U, b, Wd, bd, n_warm: int = T_IN):
    """Full unsharded reference inputs -> list of 8 per-core input maps."""
    W_aug = to_bf16(np.concatenate([W, b[None, :]], axis=0))  # [65, 4096]
    U_l = _u_layout(U)
    # decode recurrence fold: x_{t+1} = h_t@Wd + bd, so
    # z = x@W + h@U + b = h@(U + Wd@W) + (b + bd@W)
    Uf_l = _u_layout(
        (U + Wd.astype(np.float64) @ W.astype(np.float64)).astype(np.float32)
    )
    bfold = (b + bd.astype(np.float64) @ W.astype(np.float64)).astype(np.float32)
    # bf[gi*1024 + u*128 + p] -> [p, gi*KT+u]
    bf_l = np.ascontiguousarray(
        bfold.reshape(4, KT, 128).transpose(2, 0, 1).reshape(128, 4 * KT)
    )
    Wd_l = to_bf16(Wd.reshape(KT, 128, FEAT).transpose(1, 0, 2))  # [128, KT, 64]
    bd_l = np.ascontiguousarray(bd.astype(np.float32)[:, None])  # [64, 1]

    in_maps = []
    for c in range(N_CORES):
        xc = inputs[c * BC : (c + 1) * BC, :n_warm, :]  # [BC, T, F]
        xt = xc.transpose(1, 2, 0)  # [T, F, BC]
        xt_aug = np.concatenate(
            [xt, np.ones((n_warm, 1, BC), np.float32)], axis=1
        )  # [T, 65, BC]
        in_maps.append(
            {
                "xt": to_bf16(xt_aug),
                "U": U_l,
                "Uf": Uf_l,
                "W": W_aug,
                "Wd": Wd_l,
                "bd": bd_l,
                "bf": bf_l,
            }
        )
    return in_maps


def assemble_output(results, n_dec: int = OUT_STEPS):
    """Per-core outT [n_dec, 64, BC] -> full [B, n_dec, 64]."""
    outs = []
    for c in range(N_CORES):
        o = np.asarray(results[c]["outT"])  # [n_dec, FEAT, BC] bf16
        outs.append(o.transpose(2, 0, 1))  # [BC, n_dec, FEAT]
    return np.ascontiguousarray(np.concatenate(outs, axis=0).astype(np.float32))


_NC_CACHE = {}


def kernel(inputs, W, U, b, Wd, bd):
    inputs = np.asarray(inputs, dtype=np.float32)
    W = np.asarray(W, dtype=np.float32)
    U = np.asarray(U, dtype=np.float32)
    b = np.asarray(b, dtype=np.float32)
    Wd = np.asarray(Wd, dtype=np.float32)
    bd = np.asarray(bd, dtype=np.float32)
    assert inputs.shape == (B, T_IN, FEAT), inputs.shape

    # the folded decode bias b + bd@W enters via extra ACT bias operands;
    # when it is exactly zero (the spec fills b and bd with zeros) build
    # the lean variant with warm-identical fused activations instead
    bfold = b.astype(np.float64) + bd.astype(np.float64) @ W.astype(np.float64)
    use_bias = bool(np.any(np.abs(bfold) > 0))
    key = ("nc", use_bias)
    if key not in _NC_CACHE:
        _NC_CACHE[key] = build_lstm(T_IN, OUT_STEPS, use_bias=use_bias)
    nc = _NC_CACHE[key]
    _NC_CACHE["nc"] = nc

    in_maps = prep_in_maps(inputs, W, U, b, Wd, bd)
    res = run_bass_kernel_spmd(nc, in_maps, core_ids=list(range(N_CORES)))
    return assemble_output(res.results)

